# Initial kernel scaffold
#
"""Trainium2 Bass kernel for nn_ClusteringLayer (greedy per-cacheline clustering).

Contract: kernel(x) takes the FULL input (64,256,56,56) fp32 and returns the
FULL output, sharding the 802816 cachelines across 8 NeuronCores internally.

Algorithm (per 64-element cacheline, vectorized across 128 partitions x G
lines/partition): single ascending pass over positions s=0..62. A position's
state is encoded in the value itself:
  clean value x      -> not yet matched (and, once s is reached, a base)
  tagged value b*2^60 -> matched to base value b (exact exponent shift)
Per step s (suffix i>s):
  xp   = XO[s] + 1e25*is_tagged(XO[s])   (tagged cols never match anything)
  d    = XO[i] - xp                      (clean: exactly x_i - x_s)
  newly= |d| < 0.1                       (exact fp32, same rounding as ref;
                                          already-matched i are tagged-huge so
                                          never re-match -> first match wins)
  XO[i] <- (x_s * 2^60) where newly      (copy_predicated)
Final pass untags: XO = XO * 2^-60 where |XO| large. Unmatched keep x.
Input stats (fixed seed): min|x|=7.5e-8 -> min tag 8.6e10 >> detect thr 1e6;
max|x|=5.42 -> tag^2 = 3.9e37 < fp32 max. No zeros in input.
"""

from contextlib import ExitStack

import numpy as np

import concourse.bass as bass
import concourse.tile as tile
from concourse import mybir
from concourse._compat import with_exitstack
from concourse.bass_utils import run_bass_kernel_spmd

N_CORES = 8
CL = 64
FULL_SHAPE = (64, 256, 56, 56)
N_LINES = int(np.prod(FULL_SHAPE)) // CL  # 802816
LINES_PER_CORE = N_LINES // N_CORES  # 100352
THRESH = float(np.float32(0.1))
TAG = float(2.0**60)
UNTAG = float(2.0**-60)
SQ_THR = 1.0e12  # XO^2 >= this <=> tagged (clean^2 <= ~30, tagged^2 >= 7e21)
BIGOFF = 1.0e25  # poison offset for tagged source columns
PADVAL = 1.0e30  # pad column value (never matches)
ABS_MASK = 0x7FFFFFFF
THRESH_BITS = int(np.float32(0.1).view(np.int32))  # |d| < T as integer compare
DETECT_BITS = int(np.float32(1.0e6).view(np.int32))  # |XO| >= 1e6 <=> tagged
F32 = mybir.dt.float32
U8 = mybir.dt.uint8
Alu = mybir.AluOpType
Act = mybir.ActivationFunctionType


def _bcast(col_ap: bass.AP, span: int) -> bass.AP:
    """View a (P, G) column AP as (P, G, span) with stride-0 innermost dim."""
    ap_rows = [list(r) for r in col_ap.ap]
    return bass.AP(
        tensor=col_ap.tensor,
        offset=col_ap.offset,
        ap=ap_rows + [[0, span]],
    )


@with_exitstack
def _cluster_kernel(
    ctx: ExitStack,
    tc: tile.TileContext,
    out_ap: bass.AP,
    in_ap: bass.AP,
    G: int,
    n_tiles: int,
    bufs: int = 3,
):
    nc = tc.nc
    lines_per_tile = 128 * G
    W = CL + 2  # pad: col 64 = PADVAL (even-span target), col 65 unused

    xpool = ctx.enter_context(tc.tile_pool(name="xpool", bufs=bufs))
    tpool = ctx.enter_context(tc.tile_pool(name="tpool", bufs=bufs))
    cpool = ctx.enter_context(tc.tile_pool(name="cpool", bufs=4 * bufs))

    def make_state(t):
        r0 = t * lines_per_tile
        src = in_ap[r0 : r0 + lines_per_tile, :].rearrange("(p g) c -> p g c", p=128)
        XO = xpool.tile([128, G, W], F32, tag=f"xo{t % 2}")
        nc.sync.dma_start(out=XO[:, :, :CL], in_=src)
        nc.vector.memset(XO[:, :, CL : CL + 2], PADVAL)
        D = tpool.tile([128, G, W], F32, tag=f"d{t % 2}")
        A = tpool.tile([128, G, W], F32, tag=f"a{t % 2}")
        NW = tpool.tile([128, G, W], U8, tag=f"nw{t % 2}")
        return r0, XO, D, A, NW

    def emit_step(state, s):
        r0, XO, D, A, NW = state
        if True:
            colXO = XO[:, :, s]
            # c1 = 1 if col s is tagged (XO^2 >= 1e12)
            sq = cpool.tile([128, G], F32, tag="sq")
            nc.vector.tensor_tensor(out=sq[:], in0=colXO, in1=colXO, op=Alu.mult)
            c1 = cpool.tile([128, G], F32, tag="c1")
            nc.vector.tensor_scalar(
                out=c1[:], in0=sq[:], scalar1=SQ_THR, scalar2=None, op0=Alu.is_ge
            )
            xp = cpool.tile([128, G], F32, tag="xp")
            nc.vector.scalar_tensor_tensor(
                out=xp[:], in0=c1[:], scalar=BIGOFF, in1=colXO, op0=Alu.mult, op1=Alu.add
            )
            dcol = cpool.tile([128, G], F32, tag="dc")
            nc.vector.tensor_scalar(
                out=dcol[:], in0=colXO, scalar1=TAG, scalar2=None, op0=Alu.mult
            )

            rspan = CL - 1 - s  # real suffix [s+1, 64)
            espan = rspan + (rspan & 1)  # even span for the 2x-mode compare
            a, b = s + 1, s + 1 + espan
            Ds = D[:, :, a:b]
            As = A[:, :, a:b]
            nc.vector.tensor_tensor(
                out=Ds, in0=XO[:, :, a:b], in1=_bcast(xp[:, :], espan), op=Alu.subtract
            )
            nc.scalar.activation(As, Ds, Act.Abs)
            # newly = |d| < T (exact fp32, same rounding as the reference)
            nc.vector.tensor_scalar(
                out=NW[:, :, a:b],
                in0=As,
                scalar1=THRESH,
                scalar2=None,
                op0=Alu.is_lt,
            )
            nc.vector.copy_predicated(
                out=XO[:, :, a:b],
                mask=NW[:, :, a:b],
                data=_bcast(dcol[:, :], espan),
            )

    def emit_tail(state, t):
        r0, XO, D, A, NW = state
        # untag: where XO^2 >= 1e12, XO *= 2^-60
        SQT = tpool.tile([128, G, W], F32, tag=f"a{t % 2}")  # reuse a slot
        nc.vector.tensor_tensor(
            out=SQT[:, :, :CL], in0=XO[:, :, :CL], in1=XO[:, :, :CL], op=Alu.mult
        )
        MT = tpool.tile([128, G, W], U8, tag=f"nw{t % 2}")
        nc.vector.tensor_scalar(
            out=MT[:, :, :CL], in0=SQT[:, :, :CL], scalar1=SQ_THR, scalar2=None,
            op0=Alu.is_ge,
        )
        SCL = tpool.tile([128, G, W], F32, tag=f"d{t % 2}")  # reuse d slot
        nc.vector.tensor_scalar(
            out=SCL[:, :, :CL], in0=XO[:, :, :CL], scalar1=UNTAG, scalar2=None,
            op0=Alu.mult,
        )
        nc.vector.copy_predicated(
            out=XO[:, :, :CL], mask=MT[:, :, :CL], data=SCL[:, :, :CL]
        )
        dst = out_ap[r0 : r0 + lines_per_tile, :].rearrange("(p g) c -> p g c", p=128)
        nc.sync.dma_start(out=dst, in_=XO[:, :, :CL])

    # Process tiles in pairs, interleaving the two tiles' steps in program
    # order so one tile's DVE work fills the other's ACT round-trip.
    assert n_tiles % 2 == 0
    for tp in range(n_tiles // 2):
        tA, tB = 2 * tp, 2 * tp + 1
        stA = make_state(tA)
        stB = make_state(tB)
        for s in range(CL - 1):
            emit_step(stA, s)
            emit_step(stB, s)
        emit_tail(stA, tA)
        emit_tail(stB, tB)


def _split_multi_waits(nc: bass.Bass, max_waits: int = 1) -> None:
    """walrus CoreV3 codegen rejects instructions with more than one or two
    sync-wait conditions ("Too many sync wait commands"). Split extra waits
    onto single-wait NOPs inserted just before the instruction (same engine,
    same block) — semantically identical for monotonic semaphores."""

    def walk(blocks):
        for bb in blocks:
            yield bb
            sub = getattr(bb, "blocks", None)
            if sub:
                yield from walk(sub)

    for fn in nc.m.functions:
        for bb in walk(fn.blocks):
            out = []
            changed = False
            for inst in bb.instructions:
                si = inst.sync_info
                if si is not None and si.on_wait and len(si.on_wait) > max_waits:
                    waits = list(si.on_wait)
                    head, tail = waits[:-max_waits], waits[-max_waits:]
                    for k, w in enumerate(head):
                        out.append(
                            mybir.InstNoOp(
                                name=f"{inst.name}-w{k}",
                                engine=inst.engine,
                                bass_nofuse=True,
                                sync_info=mybir.SyncInfo(on_wait=[w], on_update=[]),
                            )
                        )
                    inst.sync_info = mybir.SyncInfo(
                        on_wait=tail, on_update=list(si.on_update)
                    )
                    changed = True
                out.append(inst)
            if changed:
                bb.instructions = out


def build_program(
    lines_per_core: int = LINES_PER_CORE, G: int = 49, bufs: int = 2
) -> bass.Bass:
    assert lines_per_core % (128 * G) == 0
    n_tiles = lines_per_core // (128 * G)
    nc = bass.Bass("TRN2", target_bir_lowering=False, debug=False)
    xin = nc.declare_dram_parameter("xin", [lines_per_core, CL], F32, isOutput=False)
    yout = nc.declare_dram_parameter("yout", [lines_per_core, CL], F32, isOutput=True)
    with tile.TileContext(nc) as tc:
        _cluster_kernel(tc, yout, xin, G, n_tiles, bufs=bufs)
    _split_multi_waits(nc)
    return nc


_PROGRAM_CACHE: dict = {}


def _get_program(lines_per_core: int, G: int, bufs: int = 2) -> bass.Bass:
    key = (lines_per_core, G, bufs)
    if key not in _PROGRAM_CACHE:
        _PROGRAM_CACHE[key] = build_program(lines_per_core, G, bufs)
    return _PROGRAM_CACHE[key]


def run_sharded(flat_lines: np.ndarray, G: int = 49, trace: bool = False, bufs: int = 2):
    """flat_lines: (n_lines, 64) fp32 with n_lines divisible by N_CORES*128*G.
    Returns (out_lines, BassKernelResults)."""
    n_lines = flat_lines.shape[0]
    lines_per_core = n_lines // N_CORES
    nc = _get_program(lines_per_core, G, bufs)
    in_maps = [
        {"xin": np.ascontiguousarray(flat_lines[c * lines_per_core : (c + 1) * lines_per_core])}
        for c in range(N_CORES)
    ]
    res = run_bass_kernel_spmd(nc, in_maps, list(range(N_CORES)), trace=trace)
    out = np.concatenate([res.results[c]["yout"] for c in range(N_CORES)], axis=0)
    return out, res


def kernel(x: np.ndarray) -> np.ndarray:
    x = np.ascontiguousarray(x, dtype=np.float32)
    flat = x.reshape(-1, CL)
    out, _ = run_sharded(flat, G=49, trace=False)
    return out.reshape(FULL_SHAPE).astype(np.float32)



# revision 19
# speedup vs baseline: 1.0067x; 1.0067x over previous
"""Trainium2 Bass kernel for nn_ClusteringLayer (greedy per-cacheline clustering).

Contract: kernel(x) takes the FULL input (64,256,56,56) fp32 and returns the
FULL output, sharding the 802816 cachelines across 8 NeuronCores internally.

Algorithm: pure value propagation, no tags. For each 64-element line, ascending
position s: every later element within THRESHOLD of XO[s] is overwritten with
XO[s]. This reproduces the reference greedy clustering exactly because
 (a) a matched element holds its base's value, and bases are pairwise >= T
     apart, so a matched element never re-matches a different base;
 (b) re-matching the same base value is idempotent;
 (c) a matched element acting as a source can only "match" targets that the
     same base already matched at an earlier step (same compare, same values).

Precision: everything runs in fp16 (values, differences, masks). CPU
simulation of the exact fp16 dynamics gives rel err 3.5e-3 vs the fp32
reference (budget 2e-2). fp16 enables the DVE 2x_1p / 4x perf modes.

Layout: each SBUF tile is [128 partitions, 64 positions, G lines] (position-
major, lines innermost). The step-s source XO[:, s, :] broadcast over the
suffix positions is a stride-0 *middle* dim view; every operand keeps a
contiguous stride-1 innermost dim of G fp16 elements, so slice offsets are
always 4B-aligned and the packed 2-elem/cycle DVE modes stay eligible.
The host pre-transposes each [G,64] line block to [64,G] (free) so DMA is
fully contiguous.
"""

from contextlib import ExitStack

import numpy as np

import concourse.bass as bass
import concourse.tile as tile
from concourse import mybir
from concourse._compat import with_exitstack
from concourse.bass_utils import run_bass_kernel_spmd

import numpy as _np
from concourse.dve_spec import Spec, Src0, Src1, C0, maxx, select
from concourse import dve_ops as _dve_ops
from concourse.dve_ops import DveOp, OPS

# Custom DVE op: one instruction per clustering step.
#   out = select(|in0 - in1| < s0, in0, in1)
# in0 = broadcast source column (3D AP is fine in the TTSS struct's primary
# stream; the STT struct hit "ISA wrong length" in walrus), in1 = XO suffix
# coalesced to rank 2, s0 = THRESHOLD.
_SNAP_NAME = "CLUSTER_SNAP_ANT"
if _SNAP_NAME not in _dve_ops._SUB_OPCODE_FOR_NAME:
    _d = Src0 - Src1
    CLUSTER_SNAP_ANT = DveOp(
        _SNAP_NAME,
        Spec(
            body=select(maxx(_d, Src1 - Src0) < C0, Src0, Src1),
            reference=lambda in0, in1, s0, s1, imm2: _np.where(
                _np.abs(in0 - in1) < s0, in0, in1
            ),
        ),
        subdim=False,
        uops_sha={"v3": "558fabb0c0b0e0e8"},
    )
    _idx = len(OPS)
    OPS.append(CLUSTER_SNAP_ANT)
    _dve_ops._SUB_OPCODE_FOR_NAME[_SNAP_NAME] = _dve_ops._CUSTOM_DVE_ROW_BASE + _idx
    assert _dve_ops._SUB_OPCODE_FOR_NAME[_SNAP_NAME] < 0x20
else:
    CLUSTER_SNAP_ANT = next(op for op in OPS if op.name == _SNAP_NAME)

N_CORES = 8
CL = 64
FULL_SHAPE = (64, 256, 56, 56)
N_LINES = int(np.prod(FULL_SHAPE)) // CL  # 802816
LINES_PER_CORE = N_LINES // N_CORES  # 100352
THRESH = float(np.float32(0.1))
F16 = mybir.dt.float16
U16 = mybir.dt.uint16
Alu = mybir.AluOpType
Act = mybir.ActivationFunctionType
# ACT-side mask: Relu((T - |d|) * S) -> u16 is exactly (|d| < T) for fp16 |d|:
# the largest fp16 below T is T-2.44e-5, giving 1.6 -> >=1 after conversion;
# fp16 values above T go negative -> relu -> 0; T itself is not representable.
MASK_SCALE = 65536.0
MASK_BIAS = MASK_SCALE * float(np.float32(0.1))


def _bcast_mid(col_ap: bass.AP, span: int) -> bass.AP:
    """View a (P, G) column AP as (P, span, G) with stride-0 middle dim."""
    ap_rows = [list(r) for r in col_ap.ap]
    assert len(ap_rows) == 2
    return bass.AP(
        tensor=col_ap.tensor,
        offset=col_ap.offset,
        ap=[ap_rows[0], [0, span], ap_rows[1]],
    )


@with_exitstack
def _cluster_kernel(
    ctx: ExitStack,
    tc: tile.TileContext,
    out_ap: bass.AP,
    in_ap: bass.AP,
    G: int,
    n_tiles: int,
    bufs: int = 2,
    variant: str = "cpred",
    gpsimd_sub_steps: int = 0,
):
    nc = tc.nc

    xpool = ctx.enter_context(tc.tile_pool(name="xpool", bufs=bufs))
    dpool = ctx.enter_context(tc.tile_pool(name="dpool", bufs=bufs))
    mpool = ctx.enter_context(tc.tile_pool(name="mpool", bufs=bufs))
    BIAS = None
    if variant == "hybrid":
        cpool = ctx.enter_context(tc.tile_pool(name="cpool", bufs=1))
        BIAS = cpool.tile([128, 1], mybir.dt.float32, tag="bias")
        nc.vector.memset(BIAS[:], MASK_BIAS)

    def make_state(t):
        src = in_ap[t * 128 : (t + 1) * 128, :].rearrange("p (c g) -> p c g", c=CL)
        XO = xpool.tile([128, CL, G], F16, tag=f"xo{t % 2}")
        nc.sync.dma_start(out=XO[:], in_=src)
        if variant == "fused":
            # staging tile: ACT materializes the broadcast source column here
            SRC = dpool.tile([128, CL, G], F16, tag=f"s{t % 2}")
            return (XO, SRC)
        D = dpool.tile([128, CL, G], F16, tag=f"d{t % 2}")
        A = dpool.tile([128, CL, G], F16, tag=f"a{t % 2}")
        # CopyPredicated's mask must be an integer dtype; uint16 keeps 2B width.
        # The arith variant multiplies the mask into fp16 data, so keep it f16.
        M = mpool.tile([128, CL, G], F16 if variant == "arith" else U16, tag=f"m{t % 2}")
        return XO, D, A, M

    def emit_step(state, s):
        XO = state[0]
        span = CL - 1 - s
        col = XO[:, s, :]
        srcv = _bcast_mid(col, span)
        if variant == "fused":
            # The CUSTOM_DVE_ANT ISA struct only encodes rank-2 APs, so the
            # stride-0 broadcast view can't feed it directly; the (otherwise
            # idle) ACT engine expands the source column into SRC instead.
            SRC = state[1]
            srcs = SRC[:, 0:span, :]
            nc.scalar.activation(srcs, srcv, Act.Copy)
            flat = XO[:, s + 1 :, :].rearrange("p a g -> p (a g)")
            nc.vector._custom_dve(
                CLUSTER_SNAP_ANT,
                out=flat,
                in0=srcs.rearrange("p a g -> p (a g)"),
                in1=flat,
                s0=THRESH,
            )
            return
        _, D, A, M = state
        Ds = D[:, 0:span, :]
        As = A[:, 0:span, :]
        Ms = M[:, 0:span, :]
        sub_eng = nc.gpsimd if s < gpsimd_sub_steps else nc.vector
        sub_eng.tensor_tensor(out=Ds, in0=XO[:, s + 1 :, :], in1=srcv, op=Alu.subtract)
        # |d| on the ACT engine; compare on ACT (relu-threshold) for most
        # steps to offload DVE, on DVE (is_lt) for the rest to balance load.
        nc.scalar.activation(As, Ds, Act.Abs)
        if variant == "hybrid" and s % 5 != 4:
            nc.scalar.activation(
                Ms, As, Act.Relu, bias=BIAS[:], scale=-MASK_SCALE
            )
        else:
            nc.vector.tensor_scalar(
                out=Ms, in0=As, scalar1=THRESH, scalar2=None, op0=Alu.is_lt
            )
        if variant in ("cpred", "hybrid"):
            nc.vector.copy_predicated(out=XO[:, s + 1 :, :], mask=Ms, data=srcv)
        else:
            # XO -= d*mask : matched elements become src + O(ulp(d)); CPU sim
            # of this exact dynamics gives rel err 3.5e-3.
            nc.vector.tensor_tensor(out=Ds, in0=Ds, in1=Ms, op=Alu.mult)
            nc.vector.tensor_tensor(
                out=XO[:, s + 1 :, :], in0=XO[:, s + 1 :, :], in1=Ds, op=Alu.subtract
            )

    def emit_tail(state, t):
        XO = state[0]
        dst = out_ap[t * 128 : (t + 1) * 128, :].rearrange("p (c g) -> p c g", c=CL)
        nc.sync.dma_start(out=dst, in_=XO[:])

    # Interleave tile pairs so independent work fills dependency stalls.
    assert n_tiles % 2 == 0
    for tp in range(n_tiles // 2):
        tA, tB = 2 * tp, 2 * tp + 1
        stA = make_state(tA)
        stB = make_state(tB)
        for s in range(CL - 1):
            emit_step(stA, s)
            emit_step(stB, s)
        emit_tail(stA, tA)
        emit_tail(stB, tB)


def _split_multi_waits(nc: bass.Bass, max_waits: int = 1) -> None:
    """walrus CoreV3 codegen rejects instructions with more than one or two
    sync-wait conditions ("Too many sync wait commands"). Split extra waits
    onto single-wait NOPs inserted just before the instruction (same engine,
    same block) — semantically identical for monotonic semaphores."""

    def walk(blocks):
        for bb in blocks:
            yield bb
            sub = getattr(bb, "blocks", None)
            if sub:
                yield from walk(sub)

    for fn in nc.m.functions:
        for bb in walk(fn.blocks):
            out = []
            changed = False
            for inst in bb.instructions:
                si = inst.sync_info
                if si is not None and si.on_wait and len(si.on_wait) > max_waits:
                    waits = list(si.on_wait)
                    head, tail = waits[:-max_waits], waits[-max_waits:]
                    for k, w in enumerate(head):
                        out.append(
                            mybir.InstNoOp(
                                name=f"{inst.name}-w{k}",
                                engine=inst.engine,
                                bass_nofuse=True,
                                sync_info=mybir.SyncInfo(on_wait=[w], on_update=[]),
                            )
                        )
                    inst.sync_info = mybir.SyncInfo(
                        on_wait=tail, on_update=list(si.on_update)
                    )
                    changed = True
                out.append(inst)
            if changed:
                bb.instructions = out


def build_program(
    lines_per_core: int = LINES_PER_CORE,
    G: int = 98,
    bufs: int = 2,
    variant: str = "cpred",
    gpsimd_sub_steps: int = 0,
) -> bass.Bass:
    assert lines_per_core % (128 * G) == 0
    n_tiles = lines_per_core // (128 * G)
    nc = bass.Bass("TRN2", target_bir_lowering=False, debug=False)
    xin = nc.declare_dram_parameter("xin", [n_tiles * 128, CL * G], F16, isOutput=False)
    yout = nc.declare_dram_parameter("yout", [n_tiles * 128, CL * G], F16, isOutput=True)
    with tile.TileContext(nc) as tc:
        _cluster_kernel(
            tc, yout, xin, G, n_tiles, bufs=bufs, variant=variant,
            gpsimd_sub_steps=gpsimd_sub_steps,
        )
    _split_multi_waits(nc)
    return nc


_PROGRAM_CACHE: dict = {}


def _get_program(lines_per_core: int, G: int, bufs: int, variant: str, gss: int) -> bass.Bass:
    key = (lines_per_core, G, bufs, variant, gss)
    if key not in _PROGRAM_CACHE:
        _PROGRAM_CACHE[key] = build_program(lines_per_core, G, bufs, variant, gss)
    return _PROGRAM_CACHE[key]


def run_sharded(
    flat_lines: np.ndarray,
    G: int = 98,
    trace: bool = False,
    bufs: int = 2,
    variant: str = "cpred",
    gpsimd_sub_steps: int = 0,
):
    """flat_lines: (n_lines, 64) fp32/fp16, n_lines divisible by N_CORES*128*G.
    Returns (out_lines fp16, BassKernelResults)."""
    n_lines = flat_lines.shape[0]
    lines_per_core = n_lines // N_CORES
    n_tiles = lines_per_core // (128 * G)
    nc = _get_program(lines_per_core, G, bufs, variant, gpsimd_sub_steps)
    x16 = flat_lines.astype(np.float16)
    in_maps = []
    for c in range(N_CORES):
        part = x16[c * lines_per_core : (c + 1) * lines_per_core]
        # [n_tiles,128,G,64] -> [n_tiles,128,64,G] position-major per partition
        xt = part.reshape(n_tiles, 128, G, CL).transpose(0, 1, 3, 2)
        in_maps.append(
            {"xin": np.ascontiguousarray(xt).reshape(n_tiles * 128, CL * G)}
        )
    res = run_bass_kernel_spmd(nc, in_maps, list(range(N_CORES)), trace=trace)
    outs = []
    for c in range(N_CORES):
        yt = res.results[c]["yout"].reshape(n_tiles, 128, CL, G)
        outs.append(yt.transpose(0, 1, 3, 2).reshape(lines_per_core, CL))
    return np.concatenate(outs, axis=0), res


def kernel(x: np.ndarray) -> np.ndarray:
    x = np.ascontiguousarray(x, dtype=np.float32)
    flat = x.reshape(-1, CL)
    out, _ = run_sharded(flat, G=98, trace=False)
    return out.reshape(FULL_SHAPE).astype(np.float32)
